# revision 1
# baseline (speedup 1.0000x reference)
"""Trainium2 Bass kernel for nn_Attention_35588099015465.

Full GQA attention layer (QKV proj + per-head RMS norm + head-indexed rotary +
causal SDPA + out proj), sharded over 8 NeuronCores as DP(batch=2) x TP(kv=4).

Key host-side algebra:
  - The reference's rotary angle depends only on the HEAD index (constant over
    positions), so rotary is a fixed orthogonal transform R_h per head.
    R commutes with RMS-norm (norm-preserving), and only the q/k angle
    DIFFERENCE matters for scores, so we fold R_{(h - h//G) * af} into Wq/bq on
    the host and apply no rotary on device at all.
  - q-side RMS norm scale and the 1/sqrt(D) softmax scale fold into a single
    per-(t,head) scalar c = rsqrt(sumsq_q + D*eps) applied to q^T.
  - Softmax skips max-subtraction: with unit-RMS q,k the logits are bounded by
    +-sqrt(128) ~= 11.4, so exp() cannot overflow in f32.
  - Row-parallel out-proj partials are summed on the host (the unshard step).

Device layout (per core: batch b = core//4, kv head j = core%4, q heads 4j..4j+3):
  qT (d, t), kT (d, t): head-dim on partitions -> scores S^T[tk, tq] directly.
  v (tk, d) natural (via v^T proj + PE transpose) feeds PV as stationary.
  P^T = exp(S^T) with tq on the free axis; Sigma via M=1 ones-matmuls
  (col-group packing is numerically broken on silicon - see memory notes).
  Broadcasts (norm scales, 1/Sigma) run on the otherwise-idle GPSIMD engine.
"""

import numpy as np
import ml_dtypes

B, T, C = 2, 2048, 2048
N_HEAD, N_KV = 16, 4
D = 128
G = N_HEAD // N_KV  # 4
EPS = 1.1920928955078125e-07
KC = C // 128  # 16 contraction chunks
MT = T // 128  # 16 row chunks
NT = T // 512  # 4 col chunks

_CACHE = {}


def build_nc(dbg=False):
    import concourse.mybir as mybir
    import concourse.tile as tile
    from concourse import bacc

    dt = mybir.dt
    f32, bf16 = dt.float32, dt.bfloat16
    AF = mybir.ActivationFunctionType

    nc = bacc.Bacc("TRN2", target_bir_lowering=False, debug=False, num_devices=8)

    xT_d = nc.declare_dram_parameter("xT", [C, T], bf16, isOutput=False)
    wq_d = nc.declare_dram_parameter("wq", [C, G * D], bf16, isOutput=False)
    wk_d = nc.declare_dram_parameter("wk", [C, D], bf16, isOutput=False)
    wv_d = nc.declare_dram_parameter("wv", [C, D], bf16, isOutput=False)
    wp_d = nc.declare_dram_parameter("wp", [G * D, C], bf16, isOutput=False)
    bqc_d = nc.declare_dram_parameter("bqc", [D, G], f32, isOutput=False)
    bkc_d = nc.declare_dram_parameter("bkc", [D, 1], f32, isOutput=False)
    bvc_d = nc.declare_dram_parameter("bvc", [D, 1], f32, isOutput=False)
    onesc_d = nc.declare_dram_parameter("onesc", [128, 1], bf16, isOutput=False)
    ident_d = nc.declare_dram_parameter("ident", [128, 128], bf16, isOutput=False)
    mask_d = nc.declare_dram_parameter("maskt", [128, 128], bf16, isOutput=False)
    out_d = nc.declare_dram_parameter("out", [T, C], f32, isOutput=True)
    if dbg:
        dqh_d = nc.declare_dram_parameter("dqh", [128, T], f32, isOutput=True)
        dkh_d = nc.declare_dram_parameter("dkh", [128, T], f32, isOutput=True)
        dv_d = nc.declare_dram_parameter("dv", [128, D], f32, isOutput=True)
        dis_d = nc.declare_dram_parameter("dis", [1, T], f32, isOutput=True)
        dp_d = nc.declare_dram_parameter("dp", [128, T], f32, isOutput=True)
        dyt_d = nc.declare_dram_parameter("dyt", [128, T], f32, isOutput=True)

    with tile.TileContext(nc) as tc:
        with (
            tc.tile_pool(name="consts", bufs=1) as cpool,
            tc.tile_pool(name="persist", bufs=1) as ppool,
        ):
            onesc = cpool.tile([128, 1], bf16, tag="onesc")
            nc.sync.dma_start(onesc[:], onesc_d[:])
            ident = cpool.tile([128, 128], bf16, tag="ident")
            nc.sync.dma_start(ident[:], ident_d[:])
            maskt = cpool.tile([128, 128], bf16, tag="maskt")
            nc.sync.dma_start(maskt[:], mask_d[:])
            bqc = cpool.tile([D, G], f32, tag="bqc")
            nc.sync.dma_start(bqc[:], bqc_d[:])
            bkc = cpool.tile([D, 1], f32, tag="bkc")
            nc.sync.dma_start(bkc[:], bkc_d[:])
            bvc = cpool.tile([D, 1], f32, tag="bvc")
            nc.sync.dma_start(bvc[:], bvc_d[:])
            biasq = cpool.tile([1, 1], f32, tag="biasq")
            nc.vector.memset(biasq[:], float(D) * EPS)
            biask = cpool.tile([1, 1], f32, tag="biask")
            nc.vector.memset(biask[:], EPS)

            # persistent across phases
            qh = [ppool.tile([128, T], bf16, tag="qh", bufs=G, name="qh") for _ in range(G)]
            kh = ppool.tile([128, T], bf16, tag="kh", name="kh")
            v_t = [ppool.tile([128, D], bf16, tag="v", bufs=MT, name="v") for _ in range(MT)]
            yT = [ppool.tile([128, T], bf16, tag="yT", bufs=G, name="yT") for _ in range(G)]

            # ---------------- Phase A: projections + norms ----------------
            with (
                tc.tile_pool(name="phA", bufs=1) as apool,
                tc.tile_pool(name="phA_ps", space="PSUM", bufs=4) as aps,
                tc.tile_pool(name="phA_ss", space="PSUM", bufs=2) as sps_pool,
                tc.tile_pool(name="phA_tp", space="PSUM", bufs=2) as tp_pool,
            ):
                xT_t = [apool.tile([128, T], bf16, tag="xT", bufs=KC, name="xTt") for _ in range(KC)]
                wq_t = [apool.tile([128, G * D], bf16, tag="wqt", bufs=KC, name="wqt") for _ in range(KC)]
                wk_t = [apool.tile([128, D], bf16, tag="wkt", bufs=KC, name="wkt") for _ in range(KC)]
                wv_t = [apool.tile([128, D], bf16, tag="wvt", bufs=KC, name="wvt") for _ in range(KC)]
                for k in range(KC):
                    nc.sync.dma_start(xT_t[k][:], xT_d[128 * k:128 * (k + 1), :])
                    nc.sync.dma_start(wq_t[k][:], wq_d[128 * k:128 * (k + 1), :])
                for k in range(KC):
                    nc.sync.dma_start(wk_t[k][:], wk_d[128 * k:128 * (k + 1), :])
                    nc.sync.dma_start(wv_t[k][:], wv_d[128 * k:128 * (k + 1), :])

                # PE warm-up during the input-DMA ramp: keeps HAM at full
                # clock so the first projection matmuls don't run at 1.2 GHz
                for w in range(72):
                    wps = aps.tile([128, 512], f32, tag="proj", bufs=4, name="wps")
                    nc.tensor.matmul(wps[:, :128], lhsT=ident[:], rhs=ident[:],
                                     start=True, stop=True)
                # per-head: project -> sumsq -> c -> broadcast-apply
                for g in range(G + 1):
                    src = apool.tile([128, T], bf16, tag="qsb", bufs=2, name="qsb")
                    dst = qh[g] if g < G else kh
                    bias_ap = bqc[:, g:g + 1] if g < G else bkc[:]
                    for n in range(NT):
                        ps = aps.tile([128, 512], f32, tag="proj", bufs=4)
                        for k in range(KC):
                            lhs = (wq_t[k][:, 128 * g:128 * (g + 1)] if g < G
                                   else wk_t[k][:])
                            nc.tensor.matmul(
                                ps[:], lhsT=lhs, rhs=xT_t[k][:, 512 * n:512 * (n + 1)],
                                start=(k == 0), stop=(k == KC - 1))
                        nc.vector.tensor_scalar_add(
                            src[:, 512 * n:512 * (n + 1)], ps[:], bias_ap)
                    sq_t = apool.tile([128, T], bf16, tag="sqt", bufs=2, name="sqt")
                    nc.vector.tensor_mul(sq_t[:], src[:], src[:])
                    srow = apool.tile([1, T], f32, tag="srow", bufs=1, name="srow")
                    for n in range(NT):
                        ssp = sps_pool.tile([1, 512], f32, tag="ss", bufs=2)
                        nc.tensor.matmul(
                            ssp[:], lhsT=onesc[:], rhs=sq_t[:, 512 * n:512 * (n + 1)],
                            start=True, stop=True)
                        if g < G:
                            nc.scalar.activation(
                                srow[:, 512 * n:512 * (n + 1)], ssp[:], AF.Sqrt,
                                bias=biasq[:], scale=1.0)
                        else:
                            nc.scalar.activation(
                                srow[:, 512 * n:512 * (n + 1)], ssp[:], AF.Sqrt,
                                bias=biask[:], scale=1.0 / float(D))
                    crow_f = apool.tile([1, T], f32, tag="crowf", bufs=2, name="crowf")
                    nc.vector.reciprocal_approx_fast(crow_f[:], srow[:])
                    # f32 broadcast keeps the norm scale exact (bf16 here would
                    # add 0.4% logit-scale noise on top of the matmul noise)
                    bc_sb = apool.tile([128, T], f32, tag="bcs", bufs=2, name="bcs")
                    nc.gpsimd.partition_broadcast(bc_sb[:], crow_f[:])
                    nc.vector.tensor_mul(dst[:], src[:], bc_sb[:])
                # preload the exp table set during phase A's ACT idle
                dume = apool.tile([1, 1], f32, tag="dume", bufs=1, name="dume")
                nc.scalar.activation(dume[:], biasq[:], AF.Exp)
                # v^T projection then PE-transpose to natural v tiles
                vT_sb = apool.tile([128, T], bf16, tag="vT", name="vT_sb")
                for n in range(NT):
                    ps = aps.tile([128, 512], f32, tag="proj", bufs=4)
                    for k in range(KC):
                        nc.tensor.matmul(
                            ps[:], lhsT=wv_t[k][:], rhs=xT_t[k][:, 512 * n:512 * (n + 1)],
                            start=(k == 0), stop=(k == KC - 1))
                    nc.vector.tensor_scalar_add(
                        vT_sb[:, 512 * n:512 * (n + 1)], ps[:], bvc[:])
                for m in range(MT):
                    tp = tp_pool.tile([128, 128], bf16, tag="vtp", bufs=2)
                    nc.tensor.transpose(tp[:], vT_sb[:, 128 * m:128 * (m + 1)], ident[:])
                    nc.vector.tensor_copy(v_t[m][:], tp[:])
                if dbg:
                    dcp = apool.tile([128, T], f32, tag="dcp", bufs=1, name="dcp")
                    nc.vector.tensor_copy(dcp[:], qh[0][:])
                    nc.sync.dma_start(dqh_d[:], dcp[:])
                    dcp2 = apool.tile([128, T], f32, tag="dcp2", bufs=1, name="dcp2")
                    nc.vector.tensor_copy(dcp2[:], kh[:])
                    nc.sync.dma_start(dkh_d[:], dcp2[:])
                    dcp3 = apool.tile([128, D], f32, tag="dcp3", bufs=1, name="dcp3")
                    nc.vector.tensor_copy(dcp3[:], v_t[0][:])
                    nc.sync.dma_start(dv_d[:], dcp3[:])

            # ---------------- Phase B: attention ----------------
            with (
                tc.tile_pool(name="phB", bufs=1) as bpool,
                tc.tile_pool(name="phB_s", space="PSUM", bufs=2) as spool,
                tc.tile_pool(name="phB_y", space="PSUM", bufs=2) as ypool,
                tc.tile_pool(name="phB_sg", space="PSUM", bufs=2) as sgpool,
            ):
                def scores_exp(g, kk, pT):
                    for half in (0, 1):
                        if (half + 1) * 1024 <= 128 * kk:
                            continue
                        lo_h = max(128 * kk, 1024 * half)
                        sp = spool.tile([128, 1024], f32, tag="s", bufs=2, name="sp")
                        for n in range(2 * half, 2 * half + 2):
                            if 512 * (n + 1) <= 128 * kk:
                                continue
                            lo = max(128 * kk, 512 * n)
                            nc.tensor.matmul(
                                sp[:, lo - 1024 * half:512 * (n + 1) - 1024 * half],
                                lhsT=kh[:, 128 * kk:128 * (kk + 1)],
                                rhs=qh[g][:, lo:512 * (n + 1)],
                                start=True, stop=True)
                        nc.scalar.activation(
                            pT[kk][:, lo_h:1024 * (half + 1)],
                            sp[:, lo_h - 1024 * half:1024],
                            AF.Exp)
                        if 1024 * half <= 128 * kk < 1024 * (half + 1):
                            # zero the masked upper-tri of the diagonal block
                            nc.vector.tensor_mul(
                                pT[kk][:, 128 * kk:128 * kk + 128],
                                pT[kk][:, 128 * kk:128 * kk + 128],
                                maskt[:])

                LOOK = 4  # next-head score/exp tiles emitted before this head's PV
                pT_all = {}
                for g in range(G):
                    pT = pT_all.setdefault(g, [
                        bpool.tile([128, T], bf16, tag="pT", bufs=KC + LOOK + 1,
                                   name="pT") for _ in range(MT)])
                    is_f = bpool.tile([1, T], f32, tag="isf", bufs=2, name="isf")
                    for kk in range(LOOK if g > 0 else 0, MT):
                        scores_exp(g, kk, pT)
                    if dbg and g == 0:
                        dpp = bpool.tile([128, T], f32, tag="dpp", bufs=1, name="dpp")
                        nc.vector.tensor_copy(dpp[:], pT[0][:])
                        nc.sync.dma_start(dp_d[:], dpp[:])
                    # DVE pre-pairing halves the M=1 Sigma-matmul streams:
                    # pr[p] = pT[2p] + pT[2p+1] (valid from 256p; the odd
                    # chunk's first 128 cols are below its diagonal, so the
                    # even chunk is copied through there)
                    pairs = []
                    for p in range(MT // 2):
                        pr = bpool.tile([128, T], bf16, tag="pr", bufs=10, name="pr")
                        le, lo_ = 256 * p, 256 * p + 128
                        nc.vector.tensor_copy(pr[:, le:lo_], pT[2 * p][:, le:lo_])
                        nc.vector.tensor_add(
                            pr[:, lo_:T], pT[2 * p][:, lo_:T], pT[2 * p + 1][:, lo_:T])
                        pairs.append(pr)
                    # second level: quads, in place (pairs[2q] already holds
                    # the correct [512q, 512q+256) prefix)
                    for q in range(MT // 4):
                        nc.vector.tensor_add(
                            pairs[2 * q][:, 512 * q + 256:T],
                            pairs[2 * q][:, 512 * q + 256:T],
                            pairs[2 * q + 1][:, 512 * q + 256:T])
                    if g + 1 < G:
                        pT_next = pT_all.setdefault(g + 1, [
                            bpool.tile([128, T], bf16, tag="pT", bufs=KC + LOOK + 1,
                                       name="pT") for _ in range(MT)])
                        for kk in range(LOOK):
                            scores_exp(g + 1, kk, pT_next)
                    # Sigma + PV per tq chunk
                    for n in range(NT):
                        sgp = sgpool.tile([1, 512], f32, tag="sg", bufs=2)
                        yp = ypool.tile([128, 512], f32, tag="y", bufs=2)
                        qlist = [q for q in range(MT // 4) if 512 * q < 512 * (n + 1)]
                        for i, q in enumerate(qlist):
                            lo = max(512 * q, 512 * n)
                            nc.tensor.matmul(
                                sgp[:, lo - 512 * n:512], lhsT=onesc[:],
                                rhs=pairs[2 * q][:, lo:512 * (n + 1)],
                                start=(i == 0), stop=(i == len(qlist) - 1))
                        kmax = 4 * n + 3
                        for kk in range(kmax + 1):
                            lo = max(128 * kk, 512 * n)
                            nc.tensor.matmul(
                                yp[:, lo - 512 * n:512], lhsT=v_t[kk][:],
                                rhs=pT[kk][:, lo:512 * (n + 1)],
                                start=(kk == 0), stop=(kk == kmax))
                        nc.vector.reciprocal_approx_fast(
                            is_f[:, 512 * n:512 * (n + 1)], sgp[:])
                        bcn = bpool.tile([128, 512], f32, tag="bcn", bufs=2, name="bcn")
                        nc.gpsimd.partition_broadcast(
                            bcn[:], is_f[:, 512 * n:512 * (n + 1)])
                        # fused evac: yT = (P@V psum) * broadcast(1/Sigma)
                        nc.vector.tensor_mul(
                            yT[g][:, 512 * n:512 * (n + 1)], yp[:], bcn[:])
                    if dbg and g == 0:
                        nc.sync.dma_start(dis_d[:], is_f[:])
                        dyt = bpool.tile([128, T], f32, tag="dyt", bufs=1, name="dyt")
                        nc.vector.tensor_copy(dyt[:], yT[0][:])
                        nc.sync.dma_start(dyt_d[:], dyt[:])

            # ---------------- Phase C: scale by 1/Sigma + out proj ----------------
            with (
                tc.tile_pool(name="phC", bufs=1) as cpool2,
                tc.tile_pool(name="phC_o", space="PSUM", bufs=4) as opool,
            ):
                wp_t = [cpool2.tile([128, C], bf16, tag="wpt", bufs=G, name="wpt") for _ in range(G)]
                for g in range(G):
                    nc.sync.dma_start(wp_t[g][:], wp_d[128 * g:128 * (g + 1), :])
                out_sb = [cpool2.tile([128, C], f32, tag="osb", bufs=3, name="osb") for _ in range(MT)]
                for m in range(MT):
                    for cn in range(NT):
                        op = opool.tile([128, 512], f32, tag="o", bufs=4)
                        for g in range(G):
                            nc.tensor.matmul(
                                op[:], lhsT=yT[g][:, 128 * m:128 * (m + 1)],
                                rhs=wp_t[g][:, 512 * cn:512 * (cn + 1)],
                                start=(g == 0), stop=(g == G - 1))
                        if cn % 2 == 0:
                            nc.scalar.copy(out_sb[m][:, 512 * cn:512 * (cn + 1)], op[:])
                        else:
                            nc.vector.tensor_copy(
                                out_sb[m][:, 512 * cn:512 * (cn + 1)], op[:])
                    nc.sync.dma_start(out_d[128 * m:128 * (m + 1), :], out_sb[m][:])

    nc.finalize()
    return nc


def host_inputs(x, Wq, bq, Wkv, bkv, Wproj):
    bf16 = ml_dtypes.bfloat16
    af = (1.0 / 1024.0) ** np.linspace(0.0, 1.0, D // 4, dtype=np.float32)
    af = np.concatenate([af, np.zeros(D // 4, dtype=np.float32)])  # (64,)
    onesc = np.ones((128, 1), dtype=bf16)
    ident = np.eye(128, dtype=np.float32).astype(bf16)
    p = np.arange(128)
    maskt = np.where(p[None, :] >= p[:, None], 1.0, 0.0).astype(bf16)

    xTs = [np.ascontiguousarray(x[b].T).astype(bf16) for b in range(B)]
    in_maps = []
    for core in range(8):
        b, j = core // 4, core % 4
        wq_parts, bq_parts = [], []
        for g in range(G):
            h = G * j + g
            th = (h - j) * af
            cth, sth = np.cos(th).astype(np.float32), np.sin(th).astype(np.float32)
            R = np.zeros((D, D), np.float32)
            i = np.arange(64)
            R[i, i] = cth
            R[i, 64 + i] = sth
            R[64 + i, i] = -sth
            R[64 + i, 64 + i] = cth
            wq_parts.append(Wq[:, h * D:(h + 1) * D] @ R.T)
            bq_parts.append(bq[h * D:(h + 1) * D] @ R.T)
        in_maps.append({
            "xT": xTs[b],
            "wq": np.concatenate(wq_parts, axis=1).astype(bf16),
            "wk": Wkv[:, j * D:(j + 1) * D].astype(bf16),
            "wv": Wkv[:, N_KV * D + j * D:N_KV * D + (j + 1) * D].astype(bf16),
            "wp": Wproj[G * D * j:G * D * (j + 1), :].astype(bf16),
            "bqc": np.stack(bq_parts, axis=1).astype(np.float32),  # (D, G)
            "bkc": bkv[j * D:(j + 1) * D].reshape(D, 1).astype(np.float32),
            "bvc": bkv[N_KV * D + j * D:N_KV * D + (j + 1) * D].reshape(D, 1).astype(np.float32),
            "onesc": onesc,
            "ident": ident,
            "maskt": maskt,
        })
    return in_maps


def assemble(parts, bproj):
    out = np.empty((B, T, C), np.float32)
    for b in range(B):
        out[b] = parts[4 * b] + parts[4 * b + 1] + parts[4 * b + 2] + parts[4 * b + 3]
        out[b] += bproj[None, :]
    return out


def kernel(x, mask, Wq, bq, Wkv, bkv, Wproj, bproj):
    from concourse.bass_utils import run_bass_kernel_spmd

    x = np.asarray(x, np.float32)
    in_maps = host_inputs(
        x, np.asarray(Wq, np.float32), np.asarray(bq, np.float32),
        np.asarray(Wkv, np.float32), np.asarray(bkv, np.float32),
        np.asarray(Wproj, np.float32))
    if "nc" not in _CACHE:
        _CACHE["nc"] = build_nc()
    res = run_bass_kernel_spmd(_CACHE["nc"], in_maps, list(range(8)))
    parts = [res.results[c]["out"] for c in range(8)]
    return assemble(parts, np.asarray(bproj, np.float32))

